# revision 1
# baseline (speedup 1.0000x reference)
"""Trainium2 Bass kernel for ContrastiveMultiTaskLoss.

Computes, on 8 NeuronCores (SPMD, no collectives):
  loss = 1.0*mse(price) + 0.5*mse(change) + 0.3*bce(crit)
       + 0.1 * NT-Xent(z1, z2, temp=0.1)

Strategy: every core receives the full z1/z2 ([8192,256] f32) plus a
row-block of queries zq ([2048,256]) and small per-core slices for the
positive-pair / supervised partial sums.  Each core:
  - normalizes all 16384 embedding rows (norms via bn_stats on DVE,
    rnorm = exp(-0.5*ln(n2)) on ACT), casts to bf16, and PE-transposes
    them into a resident SBUF layout znT[K=2][128, 16384]
  - normalizes+transposes its 2048 query rows identically (bit-identical
    values, so the sim diagonal is exp(10*||znq_bf16||^2) ~= e^10)
  - computes its [2048, 16384] sim slice with bf16 matmuls (K=256 as 2
    accumulating steps, N=512 per PSUM bank), exponentiates each
    [128,2048] PSUM tile in place on the scalar engine (Exp, scale=10)
    with accum_out producing row sums directly
  - subtracts e^10 (self-sim), takes Ln, accumulates log-sum-exp partials
  - computes positive-pair dots and supervised loss partials
The host sums the 8 [128,8] partial tensors and applies the loss weights.

All activation funcs used (Exp/Ln/Square/Identity/Copy/Relu/Abs) live in
the single ACT table `natural_log_exp_and_others`; _pin_act_tables makes
the greedy table-selection pass pick it so no mid-kernel table reloads
are emitted.
"""

import math

import numpy as np

import concourse.bass as bass
import concourse.mybir as mybir
import concourse.tile as tile
from concourse import bacc
from concourse.bass_utils import run_bass_kernel_spmd
from concourse.masks import make_identity

F32 = mybir.dt.float32
BF16 = mybir.dt.bfloat16
AF = mybir.ActivationFunctionType

N_CORES = 8
D = 256
KH = 2           # K halves (D = 2*128)
GCOLS = 2048     # columns per sim-group (4 PSUM banks of 512 f32)
ITEMP = 10.0     # 1/temperature
W_PRICE, W_CHANGE, W_CRIT = 1.0, 0.5, 0.3
SSL_WEIGHT = 0.1
PAD_MM = 0   # zero-matmuls per round to keep PE warm (0 = off)
MUL_ENGINE = "alt"   # "dve" | "pool" | "alt" - engine for zn scale-muls
POOL_BUFS = (3, 3, 4)  # loads, stage, small
EXP_SELF = float(np.exp(10.0).astype(np.float64))  # exp(ITEMP * ||zn||^2)
ACT_TABLE = "natural_log_exp_and_others"


class Cfg:
    def __init__(self, n):
        self.n = n                       # rows in z1 (= rows in z2)
        self.two_n = 2 * n
        self.rows_q = 2 * n // N_CORES   # query rows per core
        self.n_rowtiles = self.rows_q // 128
        self.n_groups = self.two_n // GCOLS
        self.pos_rows = n // N_CORES     # pos-pair rows per core
        self.pa = self.pos_rows // 128   # chunks of 128
        self.sup_rows = n // N_CORES
        self.sa = self.sup_rows // 128


FULL = Cfg(8192)


def _pin_act_tables(nc):
    """Make the act-table pass choose the one table containing all our
    funcs (ids/contents untouched for the chosen table: we only remove our
    funcs from the *other* tables so greedy selection can't pick them)."""
    import concourse.hw_specs as hw_specs
    tabs = hw_specs.get_activation_tables(nc.m.arch)
    ours = set(tabs[ACT_TABLE])
    for name, funcs in tabs.items():
        if name != ACT_TABLE:
            funcs -= ours


def build_program(cfg, repeat=1):
    nc = bacc.Bacc("TRN2", target_bir_lowering=False, debug=False,
                   num_devices=N_CORES)
    z1_ext = nc.dram_tensor("z1", [cfg.n, D], F32, kind="ExternalInput")
    z2_ext = nc.dram_tensor("z2", [cfg.n, D], F32, kind="ExternalInput")
    zq_ext = nc.dram_tensor("zq", [cfg.rows_q, D], F32, kind="ExternalInput")
    zp1_ext = nc.dram_tensor("zp1", [cfg.pos_rows, D], F32, kind="ExternalInput")
    zp2_ext = nc.dram_tensor("zp2", [cfg.pos_rows, D], F32, kind="ExternalInput")
    sup_ext = nc.dram_tensor("sup", [6, cfg.sup_rows], F32, kind="ExternalInput")
    part_ext = nc.dram_tensor("partials", [128, 8], F32, kind="ExternalOutput")

    with tile.TileContext(nc) as tc:
        for _ in range(repeat):
            _emit(nc, tc, cfg, z1_ext, z2_ext, zq_ext, zp1_ext, zp2_ext,
                  sup_ext, part_ext)
    _pin_act_tables(nc)
    nc.compile()
    return nc


def _emit(nc, tc, cfg, z1_ext, z2_ext, zq_ext, zp1_ext, zp2_ext,
          sup_ext, part_ext):
    from contextlib import ExitStack
    ctx = ExitStack()
    with ctx:
        singles = ctx.enter_context(tc.tile_pool(name="singles", bufs=1))
        loads = ctx.enter_context(tc.tile_pool(name="loads", bufs=POOL_BUFS[0]))
        stage = ctx.enter_context(tc.tile_pool(name="stage", bufs=POOL_BUFS[1]))
        small = ctx.enter_context(tc.tile_pool(name="small", bufs=POOL_BUFS[2]))

        ident = singles.tile([128, 128], BF16, tag="ident")
        make_identity(nc, ident[:])

        _bias_tiles = {}

        def bias_const(val):
            if val not in _bias_tiles:
                t = singles.tile([128, 1], F32, tag=f"bias{len(_bias_tiles)}",
                                 name=f"bias{len(_bias_tiles)}")
                nc.vector.memset(t[:], val)
                _bias_tiles[val] = t
            return _bias_tiles[val][:]

        partials = singles.tile([128, 8], F32, tag="partials")
        nc.vector.memset(partials[:, 5:8], 0.0)
        # zero stationary for PE-pacing pad matmuls (adds 0 to PSUM):
        # keeps PE continuously busy so it stays at full clock (the HW HAM
        # window tolerates the small gap; the cost model does not)
        zeros = singles.tile([128, 128], BF16, tag="zeros")
        nc.vector.memset(zeros[:], 0.0)

        # resident transposed bf16 embeddings: znt[h][g] = [128, GCOLS]
        znt = [[singles.tile([128, GCOLS], BF16, tag=f"znt_{h}_{g}",
                             name=f"znt_{h}_{g}")
                for g in range(cfg.n_groups)] for h in range(KH)]
        # resident transposed bf16 queries: znqt[h] = [128, rows_q]
        znqt = [singles.tile([128, cfg.rows_q], BF16, tag=f"znqt_{h}",
                             name=f"znqt_{h}")
                for h in range(KH)]
        loghold = singles.tile([128, cfg.n_rowtiles], F32, tag="loghold")
        # exp(ITEMP * ||znq_bf16||^2) per query rowtile (exact diag values)
        d2q = singles.tile([128, cfg.n_rowtiles], F32, tag="d2q")
        expdq = singles.tile([128, cfg.n_rowtiles], F32, tag="expdq")
        # accumulated exp row sums, one col per (rowtile, group)
        acc_all = singles.tile([128, cfg.n_rowtiles, cfg.n_groups], F32,
                               tag="acc_all")

        # ---------------- prologue: normalize + transpose -------------
        bigtile_idx = [0]

        def norm_cast(big_rows_ap, A, out_bf):
            """Load [128, A, 256] f32 rows; write normalized bf16 to out_bf.

            norms via bn_stats/bn_aggr (DVE), rnorm = exp(-0.5 ln(256*n2m))
            computed on ACT with the ln(256) folded into the exp bias."""
            zbig = loads.tile([128, A, D], F32, tag="zbig")
            nc.sync.dma_start(out=zbig[:], in_=big_rows_ap)
            stats = small.tile([128, A, 6], F32, tag="stats")
            mv = small.tile([128, A, 2], F32, tag="mv")
            for a in range(A):
                nc.vector.bn_stats(out=stats[:, a, :], in_=zbig[:, a, :])
                nc.vector.bn_aggr(out=mv[:, a, :], in_=stats[:, a, :])
            m2 = small.tile([128, A], F32, tag="m2")
            nc.vector.tensor_mul(m2[:], mv[:, :, 0], mv[:, :, 0])
            n2m = small.tile([128, A], F32, tag="n2m")
            nc.vector.tensor_add(n2m[:], m2[:], mv[:, :, 1])
            lnn = small.tile([128, A], F32, tag="lnn")
            nc.scalar.activation(out=lnn[:], in_=n2m[:], func=AF.Ln)
            rn = small.tile([128, A], F32, tag="rn")
            nc.scalar.activation(out=rn[:], in_=lnn[:], func=AF.Exp,
                                 scale=-0.5, bias=bias_const(-0.5 * math.log(D)))
            if MUL_ENGINE == "dve":
                eng = nc.vector
            elif MUL_ENGINE == "pool":
                eng = nc.gpsimd
            else:
                eng = nc.vector if bigtile_idx[0] % 2 == 0 else nc.gpsimd
            bigtile_idx[0] += 1
            for a in range(A):
                eng.tensor_scalar_mul(out_bf[:, a, :], zbig[:, a, :],
                                      rn[:, a:a + 1])

        def transpose_block(zn_bf, A, dest_fn, psum_pool):
            """PE-transpose [128,256] chunks; batch PSUM->SBUF copies.

            Shares the main-loop PSUM slots (tag "mp") so prologue and main
            loop can interleave without address-reuse serialization.

            dest_fn(h) -> (tile, col_offset) for the A*128-wide block."""
            for h in range(KH):
                pt = psum_pool.tile([128, A * 128], BF16, tag="mp", name="pt")
                for a in range(A):
                    nc.tensor.transpose(pt[:, a * 128:(a + 1) * 128],
                                        zn_bf[:, a, h * 128:(h + 1) * 128],
                                        ident[:])
                dst, off = dest_fn(h)
                nc.vector.tensor_copy(dst[:, off:off + A * 128], pt[:])

        with tc.tile_pool(name="mpsum", bufs=2, space="PSUM") as mpsum:
            # queries first: the main loop depends on them for every group
            zqr = zq_ext.ap().rearrange("(c p) d -> p c d", p=128)
            nqchunks = cfg.rows_q // 128
            for start in range(0, nqchunks, 8):
                A = min(8, nqchunks - start)
                znb = stage.tile([128, A, D], BF16, tag="znb")
                norm_cast(zqr[:, start:start + A, :], A, znb)
                # exact self-sim ||znq_bf16||^2 via bn stats on the bf16 tile
                qstats = small.tile([128, A, 6], F32, tag="qstats")
                qmv = small.tile([128, A, 2], F32, tag="qmv")
                for a in range(A):
                    nc.vector.bn_stats(out=qstats[:, a, :], in_=znb[:, a, :])
                    nc.vector.bn_aggr(out=qmv[:, a, :], in_=qstats[:, a, :])
                qm2 = small.tile([128, A], F32, tag="qm2")
                nc.vector.tensor_mul(qm2[:], qmv[:, :, 0], qmv[:, :, 0])
                nc.vector.tensor_add(d2q[:, start:start + A], qm2[:],
                                     qmv[:, :, 1])
                transpose_block(znb, A,
                                lambda h, s=start: (znqt[h], s * 128),
                                mpsum)
            # expdq = exp(ITEMP * D * d2q_mean)
            nc.scalar.activation(out=expdq[:], in_=d2q[:], func=AF.Exp,
                                 scale=ITEMP * D)

            # keys: z1 then z2, with each group's main-loop rounds
            # emitted as soon as the group's transposed keys are complete
            # (PSUM slots are allocation-ordered, so interleaving emission
            # is what lets sim matmuls overlap the rest of the prologue)
            def main_rounds(g):
                for m in range(cfg.n_rowtiles):
                    pt = mpsum.tile([128, GCOLS], F32, tag="mp", name="mp")
                    for h in range(KH):
                        for j in range(GCOLS // 512):
                            last = h == KH - 1
                            stop = last and (PAD_MM == 0 or j > 0)
                            nc.tensor.matmul(
                                pt[:, j * 512:(j + 1) * 512],
                                lhsT=znqt[h][:, m * 128:(m + 1) * 128],
                                rhs=znt[h][g][:, j * 512:(j + 1) * 512],
                                start=(h == 0), stop=stop)
                    for pad in range(PAD_MM):
                        nc.tensor.matmul(
                            pt[:, 0:512], lhsT=zeros[:],
                            rhs=znt[0][g][:, 0:512],
                            start=False, stop=(pad == PAD_MM - 1))
                    nc.scalar.activation(out=pt[:], in_=pt[:], func=AF.Exp,
                                         scale=ITEMP,
                                         accum_out=acc_all[:, m, g:g + 1])

            chunks_done = 0
            groups_emitted = 0
            for zi, z_ext in enumerate((z1_ext, z2_ext)):
                zr = z_ext.ap().rearrange("(c p) d -> p c d", p=128)
                nchunks = cfg.n // 128
                for start in range(0, nchunks, 8):
                    A = min(8, nchunks - start)
                    znb = stage.tile([128, A, D], BF16, tag="znb")
                    norm_cast(zr[:, start:start + A, :], A, znb)
                    base_chunk = zi * nchunks + start
                    g, c = divmod(base_chunk, GCOLS // 128)
                    transpose_block(znb, A,
                                    lambda h, g=g, c=c: (znt[h][g], c * 128),
                                    mpsum)
                    chunks_done += A
                    while (groups_emitted + 1) * (GCOLS // 128) <= chunks_done:
                        main_rounds(groups_emitted)
                        groups_emitted += 1
            assert groups_emitted == cfg.n_groups

            # ---- positive pairs + supervised (tiny) ----
            _emit_pos_sup(nc, tc, cfg, zp1_ext, zp2_ext, sup_ext,
                          partials, loads, stage, small, bias_const)

        # batched epilogue: row sums per rowtile, minus exp(self), log
        rs_all = small.tile([128, cfg.n_rowtiles], F32, tag="rs_all")
        nc.vector.tensor_reduce(out=rs_all[:], in_=acc_all[:],
                                axis=mybir.AxisListType.X,
                                op=mybir.AluOpType.add)
        rsc_all = small.tile([128, cfg.n_rowtiles], F32, tag="rsc_all")
        nc.vector.tensor_sub(rsc_all[:], rs_all[:], expdq[:])
        nc.scalar.activation(out=loghold[:], in_=rsc_all[:], func=AF.Ln)
        lhdump = small.tile([128, cfg.n_rowtiles], F32, tag="lhdump")
        nc.scalar.activation(out=lhdump[:], in_=loghold[:], func=AF.Identity,
                             accum_out=partials[:, 0:1])
        nc.sync.dma_start(out=part_ext[:], in_=partials[:])


def _emit_pos_sup(nc, tc, cfg, zp1_ext, zp2_ext, sup_ext, partials,
                  loads, stage, small, bias_const):
    A = cfg.pa
    # --- positive pair partial: sum over rows of zn1 . zn2 (unscaled) ---
    zp1r = zp1_ext.ap().rearrange("(a p) d -> p a d", p=128)
    zp2r = zp2_ext.ap().rearrange("(a p) d -> p a d", p=128)
    p1 = loads.tile([128, A, D], F32, tag="p1", bufs=1)
    p2 = loads.tile([128, A, D], F32, tag="p2", bufs=1)
    nc.sync.dma_start(out=p1[:], in_=zp1r)
    nc.sync.dma_start(out=p2[:], in_=zp2r)
    prod = stage.tile([128, A, D], F32, tag="prod", bufs=1)
    nc.vector.tensor_mul(prod[:], p1[:], p2[:])
    # bn_stats means: n2{a,b} = D*(var+mean^2); dots = D*mean(prod)
    stats = small.tile([128, 3, A, 6], F32, tag="pstats")
    mv = small.tile([128, 3, A, 2], F32, tag="pmv")
    for i, src in enumerate((p1, p2, prod)):
        for a in range(A):
            nc.vector.bn_stats(out=stats[:, i, a, :], in_=src[:, a, :])
            nc.vector.bn_aggr(out=mv[:, i, a, :], in_=stats[:, i, a, :])
    m2 = small.tile([128, 2, A], F32, tag="pm2")
    nc.vector.tensor_mul(m2[:], mv[:, 0:2, :, 0], mv[:, 0:2, :, 0])
    n2ab = small.tile([128, 2, A], F32, tag="n2ab")
    nc.vector.tensor_add(n2ab[:], m2[:], mv[:, 0:2, :, 1])
    # rnorm product: exp(-0.5*(ln(n2a*D) + ln(n2b*D)))
    lnab = small.tile([128, 2, A], F32, tag="lnab")
    nc.scalar.activation(out=lnab[:], in_=n2ab[:], func=AF.Ln)
    lnsum = small.tile([128, A], F32, tag="lnsum")
    nc.vector.tensor_add(lnsum[:], lnab[:, 0, :], lnab[:, 1, :])
    rp = small.tile([128, A], F32, tag="rp")
    nc.scalar.activation(out=rp[:], in_=lnsum[:], func=AF.Exp,
                         scale=-0.5, bias=bias_const(-math.log(D)))
    # pos = dots * rp = (D*mean(prod)) * rp
    pos = small.tile([128, A], F32, tag="pos")
    nc.vector.tensor_mul(pos[:], mv[:, 2, :, 0], rp[:])
    pdump = small.tile([128, A], F32, tag="pdump")
    # accumulate D * sum(pos)
    nc.scalar.activation(out=pdump[:], in_=pos[:], func=AF.Identity,
                         scale=float(D), accum_out=partials[:, 1:2])

    # --- supervised partials ---
    S = cfg.sa
    supr = sup_ext.ap().rearrange("s (p a) -> p s a", p=128)
    sup = loads.tile([128, 6, S], F32, tag="sup", bufs=1)
    nc.sync.dma_start(out=sup[:], in_=supr)
    d8 = small.tile([128, S], F32, tag="d8")
    sdump = small.tile([128, S], F32, tag="sdump")
    nc.vector.tensor_sub(d8[:], sup[:, 0, :], sup[:, 1, :])
    nc.scalar.activation(out=sdump[:], in_=d8[:], func=AF.Square,
                         accum_out=partials[:, 2:3])
    d8b = small.tile([128, S], F32, tag="d8b")
    nc.vector.tensor_sub(d8b[:], sup[:, 2, :], sup[:, 3, :])
    nc.scalar.activation(out=sdump[:], in_=d8b[:], func=AF.Square,
                         accum_out=partials[:, 3:4])
    # bce: relu(x) - x*t + ln(1 + exp(-|x|))
    x_ap = sup[:, 4, :]
    t_ap = sup[:, 5, :]
    r8 = small.tile([128, S], F32, tag="r8")
    nc.scalar.activation(out=r8[:], in_=x_ap, func=AF.Relu)
    a8 = small.tile([128, S], F32, tag="a8")
    nc.scalar.activation(out=a8[:], in_=x_ap, func=AF.Abs)
    e8 = small.tile([128, S], F32, tag="e8")
    nc.scalar.activation(out=e8[:], in_=a8[:], func=AF.Exp, scale=-1.0)
    l8 = small.tile([128, S], F32, tag="l8")
    nc.scalar.activation(out=l8[:], in_=e8[:], func=AF.Ln, bias=1.0)
    xt8 = small.tile([128, S], F32, tag="xt8")
    nc.vector.tensor_mul(xt8[:], x_ap, t_ap)
    s1 = small.tile([128, S], F32, tag="s1")
    nc.vector.tensor_add(s1[:], r8[:], l8[:])
    s2 = small.tile([128, S], F32, tag="s2")
    nc.vector.tensor_sub(s2[:], s1[:], xt8[:])
    nc.scalar.activation(out=sdump[:], in_=s2[:], func=AF.Identity,
                         accum_out=partials[:, 4:5])


def make_in_maps(cfg, price_pred, price_target, change_pred, change_target,
                 criticality_pred, criticality_target, z1, z2):
    z1 = np.ascontiguousarray(np.asarray(z1, dtype=np.float32))
    z2 = np.ascontiguousarray(np.asarray(z2, dtype=np.float32))
    sups = [np.asarray(a, dtype=np.float32).reshape(-1) for a in
            (price_pred, price_target, change_pred, change_target,
             criticality_pred, criticality_target)]
    in_maps = []
    rq = cfg.rows_q
    pr = cfg.pos_rows
    for c in range(N_CORES):
        qstart = c * rq
        if qstart < cfg.n:
            zq = z1[qstart:qstart + rq]
        else:
            zq = z2[qstart - cfg.n:qstart - cfg.n + rq]
        sl = slice(c * pr, (c + 1) * pr)
        sup = np.stack([s[c * cfg.sup_rows:(c + 1) * cfg.sup_rows]
                        for s in sups])
        in_maps.append({
            "z1": z1, "z2": z2,
            "zq": np.ascontiguousarray(zq),
            "zp1": np.ascontiguousarray(z1[sl]),
            "zp2": np.ascontiguousarray(z2[sl]),
            "sup": np.ascontiguousarray(sup),
        })
    return in_maps


def combine(cfg, results):
    cols = np.zeros(8, dtype=np.float64)
    for r in results:
        cols += r["partials"].astype(np.float64).sum(axis=0)
    slog, sdot, sprice, schange, scrit = cols[0], cols[1], cols[2], cols[3], cols[4]
    n = float(cfg.n)
    ssl = (slog - 2.0 * ITEMP * sdot) / (2.0 * n)
    supervised = (W_PRICE * sprice + W_CHANGE * schange + W_CRIT * scrit) / n
    return np.float32(supervised + SSL_WEIGHT * ssl)


_compiled = {}


def _get_program(repeat=1):
    key = repeat
    if key not in _compiled:
        _compiled[key] = build_program(FULL, repeat=repeat)
    return _compiled[key]


def kernel(**inputs):
    nc = _get_program()
    in_maps = make_in_maps(FULL, **inputs)
    res = run_bass_kernel_spmd(nc, in_maps, list(range(N_CORES)))
    return combine(FULL, res.results)

